# revision 3
# baseline (speedup 1.0000x reference)
"""Trainium2 Bass kernel: masked-bilinear channel-mixing Conv2d.

reference math (N=4, C=96, H=W=32, O=96, K=3, PAD=1):
    p = avgpool3x3(x, count_include_pad) -> [N, C, H, W] -> [N, L, C]
    wm = weight * mask                              [O, C, C]
    y[n,l,o] = p_l^T wm_o p_l + bias[o]

Two equivalent per-channel factorizations share one device pipeline
(rows = 24 o's x 96 inner j's = 2304, tiled 18 x 128 per L-chunk):
  eigen (o_local 0..11):  S_o = wm+wm^T = V diag(lam) V^T;
    z = (U_o^T p)^2 on ACT,  pass-2 weight = sign(lam)
  direct (o_local 12..23): t = (wm_o/81)^T p;
    z = t * p (DVE, second operand is a partition-shifted copy of p),
    pass-2 weight = 1
Splitting the PSUM->SBUF drain across BOTH ACT and DVE keeps the drain
off the critical path; the PE runs M=128 pass-1 and K=128 pass-2
matmuls back-to-back (36 x N=512 per L-chunk, FWL-friendly), which is
the bf16 streaming floor for this algorithm.

Sharding: 8 cores = 2 image-pairs x 4 O-blocks.  Core c: images
(2g, 2g+1) with g=c//4, out-channels 24j..24j+23 with j=c%4.

Per-core pipeline: DMA in (xs, W1, W2, bias); PE warmup burst holds the
HAM clock at 2.4 GHz through the pooling lead-in; 3x3 box sums in bf16
(img0 on DVE, img1 on GpSimd; 1/81 and sqrt(lam)/9 scales live in W1);
p-shift copies via SBUF->SBUF DMA; then 4 L-chunks x 18 row-tiles of
pass-1 -> drain -> lagged pass-2 accumulated in PSUM; ACT adds bias on
the PSUM->SBUF output drain.
"""
import numpy as np
import ml_dtypes

import concourse.bass as bass
import concourse.bacc as bacc
import concourse.mybir as mybir
from concourse import tile
from concourse import bass_utils

C = 96
O = 96
OB = 24            # out-channels per core
L = 2048           # locations per core (2 images x 32x32)
N_CORES = 8
NE = 12            # eigen-form o's per core (first NE of the block)
ROWS = OB * C      # 2304 intermediate rows per L-chunk
NT = ROWS // 128   # 18 row-tiles
NTE = NE * C // 128  # 9 eigen tiles (NE*96 must be divisible by 128)
LAG = 4            # pass-2 lags pass-1 by this many tiles
WARMUP_MMS = 12
F32 = mybir.dt.float32
BF16 = mybir.dt.bfloat16
BF16_NP = ml_dtypes.bfloat16


def _build_kernel(nc: bass.Bass):
    xs_d = nc.dram_tensor("xs", [C, 2 * 34 * 34], BF16, kind="ExternalInput")
    w1_d = nc.dram_tensor("w1", [C, NT * 128], BF16, kind="ExternalInput")
    w2_d = nc.dram_tensor("w2", [128, NT * OB], BF16, kind="ExternalInput")
    b_d = nc.dram_tensor("bias", [128, 1], F32, kind="ExternalInput")
    y_d = nc.dram_tensor("y", [OB, L], F32, kind="ExternalOutput")

    with tile.TileContext(nc) as tc:
        with (
            tc.tile_pool(name="const", bufs=1) as cpool,
            tc.tile_pool(name="work", bufs=1) as wpool,
            tc.tile_pool(name="z", bufs=6) as zpool,
            tc.tile_pool(name="ysb", bufs=2) as ypool_sb,
            tc.tile_pool(name="tpsum", bufs=5, space="PSUM") as tpsum,
            tc.tile_pool(name="ypsum", bufs=2, space="PSUM") as ypsum,
        ):
            xs = cpool.tile([C, 2 * 1156], BF16)
            w1 = cpool.tile([C, NT * 128], BF16)
            w2 = cpool.tile([128, NT * OB], BF16)
            bias = cpool.tile([128, 1], F32)
            warm = cpool.tile([C, 512], BF16)
            zwarm = cpool.tile([C, 8], BF16)
            pt16 = cpool.tile([C, L], BF16)
            pext = cpool.tile([128, 3 * L], BF16)  # p shifted by 0/32/64

            # DMA order: image 0 gates pooling; W1 first half gates the
            # chunk-0 eigen tiles; image 1 and the rest follow.
            half = NT * 128 // 2
            nc.sync.dma_start(xs[:, 0:1156], xs_d.ap()[:, 0:1156])
            nc.sync.dma_start(w1[:, 0:half], w1_d.ap()[:, 0:half])
            nc.sync.dma_start(bias[:], b_d.ap())
            nc.sync.dma_start(w2[:], w2_d.ap())
            nc.sync.dma_start(w1[:, half:], w1_d.ap()[:, half:])
            nc.sync.dma_start(xs[:, 1156:2312], xs_d.ap()[:, 1156:2312])

            nc.vector.memset(warm[:], 0.0)
            # preload the ACT Square spline tables while DMA runs
            nc.scalar.square(zwarm[:], warm[:, 0:8])

            # PE warmup: garbage matmuls keep the HAM clock-gate open
            # through the DMA/pooling lead-in.
            wps = ypsum.tile([128, 512], F32, tag="y_ps")
            for _ in range(WARMUP_MMS):
                nc.tensor.matmul(wps[0:C, :], warm[:, 0:C], warm[:],
                                 start=True, stop=True, skip_group_check=True)

            # --- pooling: 3x3 box sums, bf16; img0 on DVE, img1 on GpSimd
            s1 = wpool.tile([C, 2 * 34 * 33], BF16)
            s2 = wpool.tile([C, 2 * 34 * 32], BF16)
            v1 = wpool.tile([C, 2 * 33 * 32], BF16)
            for i, eng in ((0, nc.vector), (1, nc.gpsimd)):
                xv = xs[:, i * 1156:(i + 1) * 1156].rearrange(
                    "c (h w) -> c h w", h=34)
                s1v = s1[:, i * 1122:(i + 1) * 1122].rearrange(
                    "c (h w) -> c h w", h=34)
                s2v = s2[:, i * 1088:(i + 1) * 1088].rearrange(
                    "c (h w) -> c h w", h=34)
                v1v = v1[:, i * 1056:(i + 1) * 1056].rearrange(
                    "c (h w) -> c h w", h=33)
                ptv = pt16[:, i * 1024:(i + 1) * 1024].rearrange(
                    "c (h w) -> c h w", h=32)
                eng.tensor_add(s1v, xv[:, :, 0:33], xv[:, :, 1:34])
                eng.tensor_add(s2v, s1v[:, :, 0:32], xv[:, :, 2:34])
                eng.tensor_add(v1v, s2v[:, 0:33, :], s2v[:, 1:34, :])
                eng.tensor_add(ptv, v1v[:, 0:32, :], s2v[:, 2:34, :])

            # --- pext: 3 partition-shifted copies of pt16 via SBUF->SBUF
            # DMA (pext_s[q] = pt16[(s+q) % 96]); per half-image so the
            # img0 copies don't wait on GpSimd's img1 pooling.
            for i in (0, 1):
                cs, ce = i * 1024, (i + 1) * 1024
                for si, s in enumerate((0, 32, 64)):
                    off = si * L
                    n0 = 96 - s
                    nc.sync.dma_start(pext[0:n0, off + cs:off + ce],
                                      pt16[s:96, cs:ce])
                    nc.sync.dma_start(pext[n0:128, off + cs:off + ce],
                                      pt16[0:32 + s, cs:ce])

            # --- main loop: 4 L-chunks x 18 row-tiles ---
            for lc in range(4):
                y_ps = ypsum.tile([128, 512], F32, tag="y_ps")
                rhs = pt16[:, lc * 512:(lc + 1) * 512]
                pend = []

                def p2(t, z):
                    nc.tensor.matmul(
                        y_ps[0:OB, :], w2[:, t * OB:(t + 1) * OB], z[:],
                        start=(t == 0), stop=(t == NT - 1),
                        skip_group_check=True,
                    )

                for t in range(NT):
                    T = tpsum.tile([128, 512], F32, tag="T")
                    nc.tensor.matmul(T[:], w1[:, t * 128:(t + 1) * 128],
                                     rhs, start=True, stop=True)
                    z = zpool.tile([128, 512], BF16, tag="z")
                    if t < NTE:
                        nc.scalar.square(z[:], T[:])
                    else:
                        si = (t - NTE) % 3
                        nc.vector.tensor_mul(
                            z[:], T[:],
                            pext[:, si * L + lc * 512:si * L + (lc + 1) * 512])
                    pend.append((t, z))
                    if len(pend) > LAG:
                        p2(*pend.pop(0))
                while pend:
                    p2(*pend.pop(0))
                y_sb = ypool_sb.tile([128, 512], F32)
                nc.scalar.activation(
                    y_sb[0:OB, :], y_ps[0:OB, :],
                    mybir.ActivationFunctionType.Identity,
                    bias=bias[0:OB, :], scale=1.0)
                nc.sync.dma_start(y_d.ap()[:, lc * 512:(lc + 1) * 512],
                                  y_sb[0:OB, :])

    return nc


_NC_CACHE = {}


def _get_nc():
    if "nc" not in _NC_CACHE:
        nc = bacc.Bacc("TRN2", target_bir_lowering=False, debug=False,
                       enable_asserts=False)
        _build_kernel(nc)
        nc.compile()
        _NC_CACHE["nc"] = nc
    return _NC_CACHE["nc"]


def _prep_shards(x, weight, mask, bias):
    wm = np.asarray(weight, np.float32) * np.asarray(mask, np.float32)
    S = wm + wm.transpose(0, 2, 1)
    lam, V = np.linalg.eigh(S)                       # [O, R], [O, C, R]
    U = V * (np.sqrt(np.abs(lam) / 2.0)[:, None, :] / 9.0)
    sgn = np.sign(lam).astype(np.float32)            # [O, R]
    wmd = wm / 81.0                                  # direct-form weights

    x16 = np.asarray(x, np.float32).astype(BF16_NP)
    xp = np.pad(x16, ((0, 0), (0, 0), (1, 1), (1, 1)))   # [4, C, 34, 34]

    w1_blocks, w2_blocks, b_blocks = [], [], []
    bsrc = np.asarray(bias, np.float32).ravel()
    for j in range(4):
        W1 = np.zeros((C, ROWS), np.float32)
        W2 = np.zeros((128, NT * OB), np.float32)
        for ol in range(OB):
            o = OB * j + ol
            base = ol * C
            if ol < NE:
                W1[:, base:base + C] = U[o]
                wcol = sgn[o]
            else:
                W1[:, base:base + C] = wmd[o]
                wcol = np.ones(C, np.float32)
            idx = base + np.arange(C)
            W2[idx % 128, (idx // 128) * OB + ol] = wcol
        bb = np.zeros((128, 1), np.float32)
        bb[0:OB, 0] = bsrc[OB * j:OB * (j + 1)]
        w1_blocks.append(W1.astype(BF16_NP))
        w2_blocks.append(W2.astype(BF16_NP))
        b_blocks.append(bb)

    xs_pairs = []
    for g in range(2):
        xsg = np.ascontiguousarray(
            xp[2 * g:2 * g + 2].transpose(1, 0, 2, 3).reshape(C, 2 * 1156))
        xs_pairs.append(xsg.astype(BF16_NP))

    in_maps = []
    for core in range(N_CORES):
        g, j = core // 4, core % 4
        in_maps.append({"xs": xs_pairs[g], "w1": w1_blocks[j],
                        "w2": w2_blocks[j], "bias": b_blocks[j]})
    return in_maps


def run_sharded(x, weight, mask, bias, **run_kwargs):
    """Run on the 8 NeuronCores; returns (y_full, BassKernelResults)."""
    nc = _get_nc()
    in_maps = _prep_shards(x, weight, mask, bias)
    res = bass_utils.run_bass_kernel_spmd(
        nc, in_maps, core_ids=list(range(N_CORES)), **run_kwargs)
    y = np.empty((4, O, 32, 32), dtype=np.float32)
    for core in range(N_CORES):
        g, j = core // 4, core % 4
        yc = res.results[core]["y"].reshape(OB, 2, 32, 32)
        y[2 * g, OB * j:OB * (j + 1)] = yc[:, 0]
        y[2 * g + 1, OB * j:OB * (j + 1)] = yc[:, 1]
    return y, res


def kernel(x, weight, mask, bias):
    y, _ = run_sharded(x, weight, mask, bias)
    return y
